# revision 2
# baseline (speedup 1.0000x reference)
"""ChannelBlock (XCiT-style cross-covariance attention + MLP w/ ECA gate) on 8 TRN2 cores.

Sharding: data-parallel over batch B=8 (1 batch element per core); all params
replicated.  Per-core problem: x (4096, 512) fp32.

v3 design (vs the 373us q/kv baseline):
  - attention via the covariance identity: logits_h = Wk_h^T (cur^T cur) Wv_h.
    Phase 1 computes ONLY COV = cur^T cur (16 MM-512/chunk, token-major -- no
    kv GEMMs, no q GEMM, no logits MMs) + the permuted channel-major transpose
    of cur (fp16).  Halves phase-1 PE work.
  - attn-apply + proj + q-projection all folded into ONE fp16 GEMM:
    x2 = cur @ (WqG), WqG = sum_h Wq_h^T G_h, G_h = attn_h^T projW_h^T.
    WqG is built at the softmax boundary from G (16 MM-512).  The fp8 q/G
    path of the baseline is gone: attention-path quantization error ~0.
  - fp16 (not bf16) for the whole attention infra (cur, curT, COV, A, G, WqG)
    and for x2/out/y: same PE/DVE speed, 8x less rounding error.
  - fc1: contraction channels 0-255 as one fp8 DoubleRow MM, channels 256-511
    as two fp16 MMs.  fc1 weights carry per-out-channel scales s_j (both
    halves), undone by the Gelu activation's per-partition scale vector.
  - fc2: fp8 DoubleRow with per-out-channel scales s_c, undone by the y-copy
    activation scale vector (replaces the global x8 scale).
  - phase-2/3 token chunks PERMUTED (chunk a = tokens {8i+a}) exactly as the
    baseline: fc2's channel-major output tile for chunk a lands on output rows
    [512a, 512a+512).  curT is written in permuted order during phase 1.
  - ECA pool via ones-vector matmuls on fc2 output tiles; last chunk pooled
    from h1 @ colsum(fc2_w) right after gelu so the gate chain overlaps the
    final fc2/y work.
  - out_d (f16) doubles as the x2 scratch; tail adds gated y via two parallel
    streams (gpsimd accum-DMA blocks 0-3, sync readback + DVE add blocks 4-7).
  - startup: x chunk 0/1 split across sy/s/g/v queues one quarter each; chunk 0
    runs LN per-quarter so COV MMs start on first-quarter arrival.
"""

import numpy as np
import ml_dtypes
from contextlib import ExitStack

import concourse.bacc as bacc
import concourse.bass as bass
import concourse.mybir as mybir
import concourse.tile as tile
from concourse.bass import ts, ds
from concourse.bass_utils import run_bass_kernel_spmd
from concourse.masks import make_identity

F32 = mybir.dt.float32
BF16 = mybir.dt.bfloat16
F16 = mybir.dt.float16
FP8 = mybir.dt.float8e4
DR = mybir.MatmulPerfMode.DoubleRow
AF = mybir.ActivationFunctionType
ALU = mybir.AluOpType
AX = mybir.AxisListType

B = 8
NTOK = 4096
C = 512
NH = 8
HD = 64
HID = 2048
NT = 8            # token chunks of 512
TCH = NTOK // NT  # 512 tokens per chunk
P = 128
LN_EPS = 1e-5
SCALE = HD ** -0.5
FC1_FULL8 = True   # fc1 contraction fully fp8-DR (vs half fp8 + half fp16)


def _build(flags):
    """Build the per-core bass program. flags: dict of adaptive bools."""
    nc = bacc.Bacc("TRN2", target_bir_lowering=False, debug=False, num_devices=B)

    x_d = nc.dram_tensor("x", (NTOK, C), F32, kind="ExternalInput").ap()
    # Wk|Wv transposed: [c, 1024] (k columns 0:512 carry the attention scale)
    wkvT_d = nc.dram_tensor("wkvT", (C, 2 * C), F16, kind="ExternalInput").ap()
    # Wq raw (row e, col c'): [512, 512]
    wq_d = nc.dram_tensor("wq", (C, C), F16, kind="ExternalInput").ap()
    projwT_d = nc.dram_tensor("projwT", (C, C), F16, kind="ExternalInput").ap()
    n8 = C if FC1_FULL8 else 2 * P
    fc1w8T_d = nc.dram_tensor("fc1w8T", (n8, HID), FP8, kind="ExternalInput").ap()
    if not FC1_FULL8:
        fc1w16T_d = nc.dram_tensor("fc1w16T", (2 * P, HID), F16,
                                   kind="ExternalInput").ap()
    fc2wT_d = nc.dram_tensor("fc2wT", (HID, C), FP8, kind="ExternalInput").ap()
    fc1b_d = nc.dram_tensor("fc1b", (P, HID // P), F32, kind="ExternalInput").ap()
    fc1s_d = nc.dram_tensor("fc1s", (P, HID // P), F32, kind="ExternalInput").ap()
    fc2b_d = nc.dram_tensor("fc2b", (P, C // P), F32, kind="ExternalInput").ap()
    fc2s_d = nc.dram_tensor("fc2s", (P, C // P), F32, kind="ExternalInput").ap()
    ecaw_d = nc.dram_tensor("ecaw", (1, 3), F32, kind="ExternalInput").ap()
    w2s_d = nc.dram_tensor("w2s", (P, HID // P), FP8, kind="ExternalInput").ap()
    fc2bsn_d = nc.dram_tensor("fc2bsn", (1, 1), F32, kind="ExternalInput").ap()
    if flags["proj_bias"]:
        projb_d = nc.dram_tensor("projb", (1, C), F16, kind="ExternalInput").ap()
    ln_d = {}
    for nm in ("ln1w", "ln1b", "ln2w", "ln2b"):
        if flags[nm]:
            ln_d[nm] = nc.dram_tensor(nm, (C,), F32, kind="ExternalInput").ap()

    out_d = nc.dram_tensor("out", (NTOK, C), F16, kind="ExternalOutput").ap()

    v = nc.vector
    g = nc.gpsimd
    s = nc.scalar
    t = nc.tensor
    sy = nc.sync

    # strided views: token n = 1024*j + 8*p + a  <->  [a][p, j, :]
    x_perm = x_d.rearrange("(j p e) c -> e p j c", e=8, p=P)
    x2_perm = out_d.rearrange("(j p e) c -> e p j c", e=8, p=P)

    with tile.TileContext(nc) as tc, ExitStack() as ctx:
        # ---------------- pools ----------------
        consts = ctx.enter_context(tc.tile_pool(name="consts", bufs=1))
        wpool = ctx.enter_context(tc.tile_pool(name="wpool", bufs=1))
        ctp = ctx.enter_context(tc.tile_pool(name="ctp", bufs=1))
        bnd = ctx.enter_context(tc.tile_pool(name="bnd", bufs=1))
        xin = ctx.enter_context(tc.tile_pool(name="xin", bufs=4))
        curp = ctx.enter_context(tc.tile_pool(name="curp", bufs=5))
        curTp = ctx.enter_context(tc.tile_pool(name="curTp", bufs=2))
        statp = ctx.enter_context(tc.tile_pool(name="statp", bufs=3))
        smp = ctx.enter_context(tc.tile_pool(name="smp", bufs=1))
        x2p = ctx.enter_context(tc.tile_pool(name="x2p", bufs=4))
        h1p = ctx.enter_context(tc.tile_pool(name="h1p", bufs=1))
        yp = ctx.enter_context(tc.tile_pool(name="yp", bufs=1))
        outp = ctx.enter_context(tc.tile_pool(name="outp", bufs=2))

        ps_t = ctx.enter_context(tc.tile_pool(name="ps_t", bufs=2, space="PSUM"))
        ps_mm = ctx.enter_context(tc.tile_pool(name="ps_mm", bufs=5, space="PSUM"))
        ps_sm = ctx.enter_context(tc.tile_pool(name="ps_sm", bufs=1, space="PSUM"))

        # ---------------- phase-1 critical DMAs first ----------------
        # x chunk 0 split into 4 quarters, one per queue, so LN1+COV start on
        # first-quarter arrival; chunk 1 right behind on the same 4 queues.
        qeng = (sy, s, g, sy)
        xbs = {}
        for nt in (0, 1):
            xbs[nt] = xin.tile([P, 4, C], F32, name=f"xb{nt}", tag="xb", bufs=3)
            for q4 in range(4):
                qeng[(q4 + 2 * nt) % 4].dma_start(
                    out=xbs[nt][:, q4, :],
                    in_=x_d[ds(nt * TCH + q4 * P, P), :])
        ident = consts.tile([P, P], F16)
        make_identity(nc, ident)
        ones_colh = consts.tile([P, 1], F16)   # lhsT for partition-sum of y tiles
        v.memset(ones_colh, 1.0)
        ones_row = consts.tile([1, P], F16)    # lhsT for broadcast outer product
        v.memset(ones_row, 1.0)
        # preload the Exp act-table off the critical path (softmax boundary)
        tiny = consts.tile([1, 1], F32)
        v.memset(tiny, 0.0)
        s.activation(out=tiny, in_=tiny, func=AF.Exp)

        ln_bc = {}
        for nm in ln_d:
            bc = wpool.tile([P, C], F32, tag=f"lnbc_{nm}")
            g.dma_start(
                out=bc,
                in_=bass.AP(tensor=ln_d[nm].tensor, offset=ln_d[nm].offset,
                            ap=[[0, P], [1, C]]),
            )
            ln_bc[nm] = bc

        curT_sb = ctp.tile([P, 4, NTOK], F16)
        curT_v = curT_sb.rearrange("p jc (a i) -> p jc a i", a=8)

        def ln_dve(src_tiles, w_bc, b_bc, apply_eng="v"):
            """LayerNorm: DVE stats + rstd; apply on DVE or ScalarE ("s")."""
            n = len(src_tiles)
            mv = statp.tile([P, n, 2], F32, tag="mv", name=f"mv{n}")
            st = statp.tile([P, 6], F32, tag="st6")
            for p in range(n):
                v.bn_stats(out=st, in_=src_tiles[p])
                v.bn_aggr(out=mv[:, p, :], in_=st)
                st = statp.tile([P, 6], F32, tag="st6")
            # rstd = 1/sqrt(var+eps) via DVE reciprocal + 2 Newton steps
            aN = statp.tile([P, n], F32, tag="veps", name=f"veps{n}")
            v.tensor_scalar_add(out=aN, in0=mv[:, :, 1], scalar1=LN_EPS)
            rstd = statp.tile([P, n], F32, tag="rstd", name=f"rstd{n}")
            v.reciprocal(out=rstd, in_=aN)
            tN = statp.tile([P, n], F32, tag="tN", name=f"tN{n}")
            uN = statp.tile([P, n], F32, tag="uN", name=f"uN{n}")
            for _ in range(2):
                v.tensor_mul(out=tN, in0=rstd, in1=rstd)
                v.tensor_mul(out=tN, in0=tN, in1=aN)
                v.tensor_scalar(out=uN, in0=tN, scalar1=-0.5, scalar2=1.5,
                                op0=ALU.mult, op1=ALU.add)
                v.tensor_mul(out=rstd, in0=rstd, in1=uN)
            nmr = None
            if apply_eng == "s":
                # scalar-engine apply: out = Identity(x*rstd + (-mu*rstd))
                nmr = statp.tile([P, n], F32, tag="nmr", name=f"nmr{n}")
                v.tensor_mul(out=nmr, in0=mv[:, :, 0], in1=rstd)
                v.tensor_scalar_mul(out=nmr, in0=nmr, scalar1=-1.0)
            curs = []
            for p in range(n):
                if w_bc is None and b_bc is None:
                    cur = curp.tile([P, TCH], F16, tag="cur", bufs=12)
                    if apply_eng == "s":
                        s.activation(out=cur, in_=src_tiles[p], func=AF.Identity,
                                     bias=nmr[:, p:p + 1], scale=rstd[:, p:p + 1])
                    else:
                        v.tensor_scalar(out=cur, in0=src_tiles[p],
                                        scalar1=mv[:, p, 0:1],
                                        scalar2=rstd[:, p:p + 1],
                                        op0=ALU.subtract, op1=ALU.mult)
                else:
                    tmp = curp.tile([P, TCH], F32, tag="curf")
                    v.tensor_scalar(out=tmp, in0=src_tiles[p],
                                    scalar1=mv[:, p, 0:1], scalar2=rstd[:, p:p + 1],
                                    op0=ALU.subtract, op1=ALU.mult)
                    cur = curp.tile([P, TCH], F16, tag="cur", bufs=12)
                    if w_bc is not None and b_bc is not None:
                        v.tensor_mul(out=tmp, in0=tmp, in1=w_bc)
                        v.tensor_add(out=cur, in0=tmp, in1=b_bc)
                    elif w_bc is not None:
                        v.tensor_mul(out=cur, in0=tmp, in1=w_bc)
                    else:
                        v.tensor_add(out=cur, in0=tmp, in1=b_bc)
                curs.append(cur)
            return curs

        # ================= PHASE 1: LN1 + COV + curT (sw-pipelined) ======
        cov_ps = [ps_mm.tile([P, TCH], F32, tag="mm", name=f"cov{ci}")
                  for ci in range(4)]

        # COV is symmetric: accumulate only upper-triangle blocks
        # (bank ci covers columns [128*ci, 512)); lower blocks are
        # reconstructed by transposition at the boundary.
        def cov_mms(curs, nt):
            if nt == NT - 1:
                # last chunk ci-major so bank ci completes early for the copy
                for ci in range(4):
                    for p in range(4):
                        t.matmul(cov_ps[ci][:, ds(P * ci, C - P * ci)],
                                 lhsT=curs[p][:, ts(ci, P)],
                                 rhs=curs[p][:, ds(P * ci, C - P * ci)],
                                 start=False, stop=(p == 3),
                                 skip_group_check=True)
            else:
                for p in range(len(curs)):
                    for ci in range(4):
                        t.matmul(cov_ps[ci][:, ds(P * ci, C - P * ci)],
                                 lhsT=curs[p][:, ts(ci, P)],
                                 rhs=curs[p][:, ds(P * ci, C - P * ci)],
                                 start=False, stop=False, skip_group_check=True)

        def cov_mms_q(cur, first):
            for ci in range(4):
                t.matmul(cov_ps[ci][:, ds(P * ci, C - P * ci)],
                         lhsT=cur[:, ts(ci, P)],
                         rhs=cur[:, ds(P * ci, C - P * ci)],
                         start=first, stop=False, skip_group_check=True)

        def transp_scatter(nt, curs):
            """Transpose 4 cur tiles, scatter into permuted curT (fp16)."""
            for cj in range(4):
                pst = ps_t.tile([P, TCH], F32, tag="pst")
                for p in range(4):
                    t.matmul(pst[:, ts(p, P)], lhsT=curs[p][:, ts(cj, P)],
                             rhs=ident, start=True, stop=True)
                # permuted write: curT[:, cj, a*512 + 64*nt + i] = cur[8i+a]
                dst = curT_v[:, cj, :, ds(64 * nt, 64)]
                src = pst.rearrange("p (i a) -> p a i", a=8)
                if cj == 1:
                    v.tensor_copy(out=dst, in_=src)
                else:
                    s.copy(out=dst, in_=src)

        for nt in range(NT):
            xb = xbs.pop(nt)
            xts = [xb[:, q, :] for q in range(4)]
            if nt == 0:
                curs = []
                for q in range(4):
                    cq = ln_dve([xts[q]], ln_bc.get("ln1w"), ln_bc.get("ln1b"),
                                apply_eng="s")
                    cov_mms_q(cq[0], first=(q == 0))
                    curs.append(cq[0])
            else:
                curs = ln_dve(xts, ln_bc.get("ln1w"), ln_bc.get("ln1b"),
                              apply_eng="s")
                cov_mms(curs, nt)
            if nt == NT - 1:
                curs_last = curs  # transposed after the boundary A MMs
            else:
                transp_scatter(nt, curs)
            # prefetch AFTER this chunk's work so the scheduler orders this
            # chunk's LN ahead of the next chunks' DMA-dependent stats
            pres = (2, 3, 4) if nt == 0 else ((nt + 4,) if nt + 4 < NT else ())
            for pre in pres:
                xbs[pre] = xin.tile([P, 4, C], F32, name=f"xb{nt}_{pre}",
                                    tag="xb", bufs=3)
                sy.dma_start(
                    out=xbs[pre],
                    in_=x_d[ds(pre * TCH, TCH), :].rearrange(
                        "(q p) c -> p q c", p=P))
            if nt == 1:
                # prefetch all weights (g/v/s queues; sy keeps feeding x)
                wkv_sb = wpool.tile([P, 4, 2 * C], F16)
                for cj in range(4):
                    (g if cj % 2 else s).dma_start(out=wkv_sb[:, cj, :],
                                                   in_=wkvT_d[ts(cj, P), :])
                wq_sb = wpool.tile([P, 4, C], F16)
                g.dma_start(out=wq_sb,
                            in_=wq_d[:, :].rearrange("(q p) c -> p q c", p=P))
                projw_sb = wpool.tile([P, 4, C], F16)
                s.dma_start(out=projw_sb,
                            in_=projwT_d[:, :].rearrange("(q p) c -> p q c", p=P))
                fc1w8_sb = wpool.tile([P, n8 // P, HID], FP8)
                g.dma_start(out=fc1w8_sb,
                            in_=fc1w8T_d[:, :].rearrange("(q p) c -> p q c", p=P))
                if not FC1_FULL8:
                    fc1w16_sb = wpool.tile([P, 2, HID], F16)
                    for q2 in range(2):
                        (g if q2 else s).dma_start(
                            out=fc1w16_sb[:, q2, :], in_=fc1w16T_d[ts(q2, P), :])
                fc2w_sb = wpool.tile([P, 16, C], FP8)
                for jc in range(4):
                    (s if jc % 2 else g).dma_start(
                        out=fc2w_sb[:, ts(jc, 4), :],
                        in_=fc2wT_d[ds(jc * 4 * P, 4 * P), :].rearrange(
                            "(q p) c -> p q c", p=P))
                fc1b_sb = wpool.tile([P, HID // P], F32)
                s.dma_start(out=fc1b_sb, in_=fc1b_d[:, :])
                fc1s_sb = wpool.tile([P, HID // P], F32)
                s.dma_start(out=fc1s_sb, in_=fc1s_d[:, :])
                fc2b_sb = wpool.tile([P, C // P], F32)
                s.dma_start(out=fc2b_sb, in_=fc2b_d[:, :])
                fc2s_sb = wpool.tile([P, C // P], F32)
                s.dma_start(out=fc2s_sb, in_=fc2s_d[:, :])
                eca_sb = wpool.tile([1, 3], F32)
                s.dma_start(out=eca_sb, in_=ecaw_d[:, :])
                w2s_sb = wpool.tile([P, HID // P], FP8)
                s.dma_start(out=w2s_sb, in_=w2s_d[:, :])
                fc2bsn_sb = wpool.tile([1, 1], F32)
                s.dma_start(out=fc2bsn_sb, in_=fc2bsn_d[:, :])
                if flags["proj_bias"]:
                    projb_sb = wpool.tile([1, C], F16)
                    s.dma_start(out=projb_sb, in_=projb_d[:, :])

        # ================= BOUNDARY: COV -> A -> logits -> softmax -> G -> WqG
        def cpy(eng, out, in_):
            if eng is s:
                s.copy(out=out, in_=in_)
            else:
                eng.tensor_copy(out=out, in_=in_)

        cov_sb = bnd.tile([P, 4, C], F16)
        for ci in range(4):
            cpy(v if ci % 2 else s,
                cov_sb[:, ci, ds(P * ci, C - P * ci)],
                cov_ps[ci][:, ds(P * ci, C - P * ci)])
        # lower-triangle blocks: COV[cj-rows, ci-cols] = COV[ci-rows, cj-cols]^T
        for ci in range(4):
            for cj in range(ci + 1, 4):
                pstx = ps_t.tile([P, TCH], F32, tag="pst")
                t.matmul(pstx[:, 0:P], lhsT=cov_sb[:, ci, ds(P * cj, P)],
                         rhs=ident, start=True, stop=True)
                cpy(v if (ci + cj) % 2 else s,
                    cov_sb[:, cj, ds(P * ci, P)], pstx[:, 0:P])
        a_ps = [ps_mm.tile([P, C], F32, tag="mm", name=f"aps{ci}")
                for ci in range(4)]
        for ci in range(4):
            for cj in range(4):
                t.matmul(a_ps[ci], lhsT=cov_sb[:, cj, ds(ci * P, P)],
                         rhs=wkv_sb[:, cj, ds(C, C)],
                         start=(cj == 0), stop=(cj == 3))
        a_sb = bnd.tile([P, 4, C], F16)
        for ci in range(4):
            cpy(v if ci % 2 else s, a_sb[:, ci, :], a_ps[ci])
        # chunk 7's transposes deferred here: they overlap the softmax/G/WqG
        # DVE+ACT chain instead of delaying the A matmuls
        transp_scatter(NT - 1, curs_last)

        logits_ps = ps_sm.tile([P, 4, HD], F32, tag="psm")
        for h in range(NH):
            hp, half = h // 2, h % 2
            rows = slice(64 * half, 64 * half + 64)
            for cj in range(4):
                t.matmul(logits_ps[rows, hp, :],
                         lhsT=wkv_sb[:, cj, ds(64 * h, 64)],
                         rhs=a_sb[:, cj, ds(64 * h, 64)],
                         start=(cj == 0), stop=(cj == 3))

        # softmax + G + WqG accumulation, pipelined per head-pair
        wqg_ps = [ps_mm.tile([P, C], F32, tag="mm", name=f"wqg{ci}")
                  for ci in range(4)]
        G_sb = bnd.tile([P, 4, C], F16)
        for hp in range(4):
            a128 = smp.tile([P, P], F16, tag="a128", bufs=2)
            for half in range(2):
                rows = slice(64 * half, 64 * half + 64)
                nm = smp.tile([P, 1], F32, tag="nm", bufs=2)
                v.tensor_reduce(out=nm[rows, :], in_=logits_ps[rows, hp, :],
                                axis=AX.X, op=ALU.max, negate=True)
                esb = smp.tile([P, 64], F32, tag="esb", bufs=2)
                ssum = smp.tile([P, 1], F32, tag="ssum", bufs=2)
                s.activation(out=esb[rows, :], in_=logits_ps[rows, hp, :],
                             func=AF.Exp, bias=nm[rows, :], scale=1.0,
                             accum_out=ssum[rows, :])
                v.reciprocal(out=ssum[rows, :], in_=ssum[rows, :])
                v.tensor_scalar_mul(out=a128[rows, ds(64 * half, 64)],
                                    in0=esb[rows, :], scalar1=ssum[rows, :])
            # G_h[e, c] = sum_d attn_h[d, e] * projwT[64h+d, c]
            gps = ps_t.tile([P, TCH], F32, tag="pst")
            for half in range(2):
                rows = slice(64 * half, 64 * half + 64)
                t.matmul(gps[rows, 0:C], lhsT=a128[rows, rows],
                         rhs=projw_sb[rows, hp, :], start=True, stop=True)
            s.copy(out=G_sb[:, hp, :], in_=gps[:, 0:C])
            # WqG[c', c] += Wq[e-block hp, c']^T @ G[e-block hp, c]
            for ci in range(4):
                t.matmul(wqg_ps[ci], lhsT=wq_sb[:, hp, ds(ci * P, P)],
                         rhs=G_sb[:, hp, :],
                         start=(hp == 0), stop=(hp == 3), skip_group_check=True)
        wqg_sb = bnd.tile([P, 4, C], F16)
        for ci in range(4):
            cpy(v if ci % 2 else s, wqg_sb[:, ci, :], wqg_ps[ci])

        # ================= PHASE 2+3: x2 = cur@WqG (+x), LN2, MLP (permuted) ==
        pool_ps = ps_sm.tile([1, C], F32, tag="psm")
        yT_sb = yp.tile([P, 4, NTOK], F16)

        def fc1_jc(jc, cur2T8, cur2T16, h1T):
            ps = ps_mm.tile([P, TCH], F32, tag="mm")
            if FC1_FULL8:
                for k in range(2):
                    t.matmul(ps, lhsT=fc1w8_sb[:, 2 * k:2 * k + 2, ts(jc, P)],
                             rhs=cur2T8[:, 2 * k:2 * k + 2, :],
                             start=(k == 0), stop=(k == 1), perf_mode=DR)
            else:
                # half the contraction (channels 0-255) in fp8 DoubleRow
                t.matmul(ps, lhsT=fc1w8_sb[:, 0:2, ts(jc, P)],
                         rhs=cur2T8[:, 0:2, :],
                         start=True, stop=False, perf_mode=DR)
                for k in range(2):
                    t.matmul(ps, lhsT=fc1w16_sb[:, k, ts(jc, P)],
                             rhs=cur2T16[:, k, :],
                             start=False, stop=(k == 1))
            s.activation(out=h1T[:, jc, :], in_=ps, func=AF.Gelu,
                         bias=fc1b_sb[:, jc:jc + 1],
                         scale=fc1s_sb[:, jc:jc + 1])

        def fc2_y(a, cc, ps):
            yslc = yT_sb[:, cc, ds(a * TCH, TCH)]
            v.tensor_scalar(out=yslc, in0=ps,
                            scalar1=fc2s_sb[:, cc:cc + 1],
                            scalar2=fc2b_sb[:, cc:cc + 1],
                            op0=ALU.mult, op1=ALU.add)
            return yslc

        def fc_block(a, cur2T8, cur2T16):
            """fc1 + fc2 with fc2's cc 0-1 interleaved k-major into fc1's
            gelu-paced stretch, so the PE isn't starved while ScalarE drains
            GELUs.  Last chunk interleaves the h1 pool matmuls instead (gate
            chain latency beats fc2 there)."""
            h1T = h1p.tile([P, 16, TCH], FP8, tag="h1T")
            last = a == NT - 1
            fc2ps = [ps_t.tile([P, TCH], F32, tag="pst", name=f"fc2ps{cc}")
                     for cc in range(2)]

            def fc2_pair(k, stop):
                for cc in range(2):
                    t.matmul(fc2ps[cc],
                             lhsT=fc2w_sb[:, 2 * k:2 * k + 2, ts(cc, P)],
                             rhs=h1T[:, 2 * k:2 * k + 2, :],
                             start=(k == 0), stop=stop, perf_mode=DR,
                             skip_group_check=True)

            def pool_h1(jc, stop):
                t.matmul(pool_ps[0:1, :], lhsT=w2s_sb[:, jc:jc + 1],
                         rhs=h1T[:, jc, :],
                         start=False, stop=stop, skip_group_check=True)

            for jc in range(4):
                fc1_jc(jc, cur2T8, cur2T16, h1T)
            for k in range(6):
                if last:
                    pool_h1(2 * k, False)
                    pool_h1(2 * k + 1, False)
                else:
                    fc2_pair(k, stop=False)
                fc1_jc(4 + 2 * k, cur2T8, cur2T16, h1T)
                fc1_jc(5 + 2 * k, cur2T8, cur2T16, h1T)
            if last:
                for jc in range(12, 16):
                    pool_h1(jc, stop=(jc == 15))
                for k in range(8):
                    fc2_pair(k, stop=(k == 7))
            else:
                fc2_pair(6, stop=False)
                fc2_pair(7, stop=True)
            ys = [fc2_y(a, cc, fc2ps[cc]) for cc in range(2)]
            for cc in range(2, 4):
                ps = ps_t.tile([P, TCH], F32, tag="pst", name=f"fc2ps{cc}")
                for k in range(8):
                    t.matmul(ps, lhsT=fc2w_sb[:, 2 * k:2 * k + 2, ts(cc, P)],
                             rhs=h1T[:, 2 * k:2 * k + 2, :],
                             start=(k == 0), stop=(k == 7), perf_mode=DR)
                ys.append(fc2_y(a, cc, ps))
            if not last:
                # pooled[i] += sum_ch y[8i+a, ch]: pre-sum the 4 cc tiles on
                # DVE, then ONE partition-reduction matmul
                ysum = x2p.tile([P, TCH], F16, tag="ysum", bufs=2)
                v.tensor_add(out=ysum, in0=ys[0], in1=ys[1])
                v.tensor_add(out=ysum, in0=ysum, in1=ys[2])
                v.tensor_add(out=ysum, in0=ysum, in1=ys[3])
                t.matmul(pool_ps[0:1, :], lhsT=ones_colh, rhs=ysum,
                         start=(a == 0), stop=False, skip_group_check=True)

        def ln_pe2(curs):
            """Transpose LN2 tiles -> fp8 blocks for DR fc1 (+fp16 if half)."""
            cur2T8 = curTp.tile([P, 4 if FC1_FULL8 else 2, TCH], FP8, tag="c2T8")
            cur2T16 = None
            if not FC1_FULL8:
                cur2T16 = curTp.tile([P, 2, TCH], F16, tag="c2T16")
            for cj in range(4):
                pst = ps_t.tile([P, TCH], F32, tag="pst")
                for p in range(4):
                    t.matmul(pst[:, ts(p, P)], lhsT=curs[p][:, ts(cj, P)],
                             rhs=ident, start=True, stop=True)
                if FC1_FULL8:
                    if cj % 2:
                        v.tensor_copy(out=cur2T8[:, cj, :], in_=pst)
                    else:
                        s.copy(out=cur2T8[:, cj, :], in_=pst)
                elif cj < 2:
                    v.tensor_copy(out=cur2T8[:, cj, :], in_=pst)
                else:
                    s.copy(out=cur2T16[:, cj - 2, :], in_=pst)
            return cur2T8, cur2T16

        pend2 = None
        # tail readback tiles; blocks 4-7 reuse the (dead) boundary tiles'
        # SBUF slots -- same [P, 4, C] f16 shape, zero extra space
        xzs = {}
        bnd_tags = {4: "cov_sb", 5: "a_sb", 6: "G_sb", 7: "wqg_sb"}
        for a in range(NT):
            x2ts = []
            for j in range(4):
                ps = ps_mm.tile([P, TCH], F32, tag="mm")
                for cj in range(4):
                    t.matmul(ps, lhsT=curT_sb[:, cj, ds(a * TCH + j * P, P)],
                             rhs=wqg_sb[:, cj, :],
                             start=(cj == 0),
                             stop=(cj == 3 and not flags["proj_bias"]))
                if flags["proj_bias"]:
                    t.matmul(ps, lhsT=ones_row, rhs=projb_sb,
                             start=False, stop=True)
                xt = xin.tile([P, C], F32, tag="xt", bufs=3)
                sy.dma_start(out=xt, in_=x_perm[a][:, j, :])
                x2t = x2p.tile([P, C], F16, tag="x2t")
                v.tensor_add(out=x2t, in0=ps, in1=xt)
                sy.dma_start(out=x2_perm[a][:, j, :], in_=x2t)
                x2ts.append(x2t)
            if a == NT - 1:
                # x2 fully written once these 4 land: issue all tail readbacks
                # now so they drain during the last two fc_blocks
                for ra in range(NT):
                    if ra < 4:
                        xz = outp.tile([P, 4, C], F16, name=f"xz{ra}",
                                       tag="xz", bufs=4)
                    else:
                        xz = bnd.tile([P, 4, C], F16, name=f"xz{ra}",
                                      tag=bnd_tags[ra])
                    (sy, s, g)[ra % 3].dma_start(
                        out=xz,
                        in_=out_d[ds(ra * TCH, TCH), :].rearrange(
                            "(cc p) c -> p cc c", p=P))
                    xzs[ra] = xz
            curs = ln_dve(x2ts, ln_bc.get("ln2w"), ln_bc.get("ln2b"))
            if pend2 is not None:
                fc_block(*pend2)
            cur2T8, cur2T16 = ln_pe2(curs)
            pend2 = (a, cur2T8, cur2T16)
        # x2 fully written: issue readbacks for the DVE-add tail blocks now so
        # they land during the final fc_block while DMA is otherwise idle
        fc_block(*pend2)

        # ================= TAIL =================
        # ----- ECA gate -----
        ppad = smp.tile([1, C + 2], F32, tag="ppad")
        v.memset(ppad, 0.0)
        s.activation(out=ppad[:, 1:C + 1], in_=pool_ps, func=AF.Identity,
                     bias=fc2bsn_sb[0:1, 0:1], scale=1.0 / NTOK)
        cv = smp.tile([1, C], F32, tag="cv")
        v.tensor_scalar_mul(out=cv, in0=ppad[0:1, 0:C], scalar1=eca_sb[0:1, 0:1])
        v.scalar_tensor_tensor(out=cv, in0=ppad[0:1, 1:C + 1], scalar=eca_sb[0:1, 1:2],
                               in1=cv, op0=ALU.mult, op1=ALU.add)
        v.scalar_tensor_tensor(out=cv, in0=ppad[0:1, 2:C + 2], scalar=eca_sb[0:1, 2:3],
                               in1=cv, op0=ALU.mult, op1=ALU.add)
        cvb = smp.tile([1, C], F16, tag="cvb")
        s.activation(out=cvb, in_=cv, func=AF.Sigmoid)
        psb = ps_t.tile([P, TCH], F32, tag="pst")
        t.matmul(psb[:, 0:C], lhsT=ones_row, rhs=cvb, start=True, stop=True)
        sB = consts.tile([P, C], F16)
        # +1 (the "y4*gate + y4" residual) folded into the broadcast evac
        s.activation(out=sB, in_=psb[:, 0:C], func=AF.Identity, bias=1.0)

        # ----- out[512a+.] = x2 + sB * yT_a -----
        # gate-muls on DVE for all blocks; the +x2 adds split between the
        # (otherwise idle) PE -- identity-matmul accumulate, ScalarE evac back
        # into the readback tile -- and DVE block-wide adds.
        for a in range(NT):
            wt = outp.tile([P, 4, C], F16, tag="wt", bufs=2)
            for cc in range(4):
                v.tensor_mul(out=wt[:, cc, :],
                             in0=yT_sb[:, cc, ds(a * TCH, TCH)], in1=sB)
            if a < 4:
                for cc in range(4):
                    zps = ps_mm.tile([P, TCH], F32, tag="mm")
                    t.matmul(zps, lhsT=ident, rhs=wt[:, cc, :],
                             start=True, stop=False)
                    t.matmul(zps, lhsT=ident, rhs=xzs[a][:, cc, :],
                             start=False, stop=True)
                    s.copy(out=xzs[a][:, cc, :], in_=zps)
            else:
                v.tensor_add(out=xzs[a], in0=xzs[a], in1=wt)
            (sy, s, g)[a % 3].dma_start(
                out=out_d[ds(a * TCH, TCH), :].rearrange(
                    "(cc p) c -> p cc c", p=P),
                in_=xzs[a])

    nc.compile()
    return nc


_CACHE = {}


def _get_program(flags):
    key = tuple(sorted(flags.items()))
    if key not in _CACHE:
        _CACHE[key] = _build(flags)
    return _CACHE[key]


def _host_prep(inputs):
    f8 = ml_dtypes.float8_e4m3
    qkv_w = np.asarray(inputs["qkv_w"], np.float32).copy()
    qkv_w[C:2 * C, :] *= SCALE  # fold attention scale into k weights
    flags = {
        "ln1w": not np.all(inputs["ln1_w"] == 1.0),
        "ln1b": bool(np.any(inputs["ln1_b"] != 0.0)),
        "ln2w": not np.all(inputs["ln2_w"] == 1.0),
        "ln2b": bool(np.any(inputs["ln2_b"] != 0.0)),
        "proj_bias": bool(np.any(inputs["proj_b"] != 0.0)),
    }
    fc1w = np.asarray(inputs["fc1_w"], np.float32)          # (HID, C)
    fc2w = np.asarray(inputs["fc2_w"], np.float32)          # (C, HID)
    s1 = 224.0 / np.maximum(np.abs(fc1w).max(axis=1), 1e-6)  # per hidden j
    s2 = 224.0 / np.maximum(np.abs(fc2w).max(axis=1), 1e-6)  # per out-ch c
    fc1ws = fc1w * s1[:, None]
    fc2ws = fc2w * s2[:, None]
    common = {
        "wkvT": np.ascontiguousarray(qkv_w[C:3 * C].T).astype(np.float16),
        "wq": np.ascontiguousarray(qkv_w[0:C]).astype(np.float16),
        "projwT": np.ascontiguousarray(
            np.asarray(inputs["proj_w"], np.float32).T).astype(np.float16),
        "fc1w8T": np.ascontiguousarray(
            fc1ws.T if FC1_FULL8 else fc1ws.T[:2 * P, :]).astype(f8),
        "fc2wT": np.ascontiguousarray(fc2ws.T).astype(f8),
        **({} if FC1_FULL8 else {
            "fc1w16T": np.ascontiguousarray(fc1ws.T[2 * P:, :])
            .astype(np.float16)}),
        "fc1b": np.ascontiguousarray(
            np.asarray(inputs["fc1_b"], np.float32).reshape(HID // P, P).T),
        "fc1s": np.ascontiguousarray((1.0 / s1).reshape(HID // P, P).T),
        "fc2b": np.ascontiguousarray(
            np.asarray(inputs["fc2_b"], np.float32).reshape(C // P, P).T),
        "fc2s": np.ascontiguousarray((1.0 / s2).reshape(C // P, P).T),
        "ecaw": np.asarray(inputs["eca_w"], np.float32).reshape(1, 3),
        "w2s": np.ascontiguousarray(
            fc2w.sum(axis=0).reshape(HID // P, P).T).astype(f8),
        "fc2bsn": np.asarray(inputs["fc2_b"], np.float32).sum()
            .reshape(1, 1) / NTOK,
    }
    if flags["proj_bias"]:
        common["projb"] = np.asarray(inputs["proj_b"], np.float32) \
            .reshape(1, C).astype(np.float16)
    for nm, key in (("ln1w", "ln1_w"), ("ln1b", "ln1_b"),
                    ("ln2w", "ln2_w"), ("ln2b", "ln2_b")):
        if flags[nm]:
            common[nm] = np.asarray(inputs[key], np.float32)
    return flags, common


def kernel(**inputs):
    flags, common = _host_prep(inputs)
    nc = _get_program(flags)
    x = np.asarray(inputs["x"], np.float32)
    in_maps = [dict(common, x=np.ascontiguousarray(x[i])) for i in range(B)]
    res = run_bass_kernel_spmd(nc, in_maps, list(range(B)))
    return np.stack([np.asarray(r["out"], np.float32) for r in res.results], axis=0)


# revision 3
# speedup vs baseline: 1.0251x; 1.0251x over previous
"""ChannelBlock (XCiT-style cross-covariance attention + MLP w/ ECA gate) on 8 TRN2 cores.

Sharding: data-parallel over batch B=8 (1 batch element per core); all params
replicated.  Per-core problem: x (4096, 512) fp32.

HW ~320us (prior baseline 373us).  Rel err 1.75e-2 (gate 2e-2).  Design:
  - attention via the covariance identity: logits_h = Wk_h^T (cur^T cur) Wv_h.
    Phase 1 computes ONLY COV = cur^T cur (upper-triangle blocks only --
    symmetric; lower blocks reconstructed by 6 tiny transposes at the
    boundary) + the permuted channel-major transpose of cur (fp16).  No kv
    GEMMs, no q GEMM, no logits MMs in phase 1.
  - attn-apply + proj + q-projection all folded into ONE fp16 GEMM:
    x2 = cur @ (WqG), WqG = sum_h Wq_h^T G_h, G_h = attn_h^T projW_h^T.
    WqG built at the softmax boundary via A = COV @ Wv^T -> logits -> softmax
    -> G -> WqG.  The fp8 q/G path of the old baseline is gone: attention-path
    quantization error ~0.  (fp8-DR x2 was tried: slower AND less accurate.)
  - fp16 (not bf16) for the whole attention infra (cur, curT, COV, A, G, WqG)
    and for x2/out/y: same PE/DVE speed, 8x less rounding error.
  - fc1 contraction fully fp8 DoubleRow; fc1/fc2 weights carry per-out-channel
    scales (224/max|row|), undone by the Gelu / y-evac per-partition scale
    vectors.  h1 fp8.  This is where the error budget is spent.
  - fc_block interleaves fc2's cc 0-1 (k-major, 2 psum banks from the ps_t
    pool) into fc1's gelu-paced stretch so the PE isn't starved while ScalarE
    drains GELUs; y evac on DVE; pool = DVE pre-sum of the 4 y tiles + one
    partition-reduction matmul.  Last chunk interleaves the h1 pool matmuls
    instead (h1 @ colsum(fc2_w)) so the ECA gate chain overlaps its fc2.
  - phase-2/3 token chunks PERMUTED (chunk a = tokens {8i+a}): fc2's
    channel-major output tile for chunk a lands directly on output rows
    [512a, 512a+512).  curT is written in permuted order during phase 1.
  - out_d (f16) doubles as the x2 scratch; all 8 tail readbacks issue right
    after the last x2 write (draining during the final two fc_blocks, 3 DMA
    queues); tail = DVE gate-muls + adds split between DVE and the idle PE
    (identity-matmul accumulate, ScalarE evac); writes across 3 queues.
  - startup: x chunks 0/1 split one quarter per queue; chunk 0 runs LN
    per-quarter so COV MMs start on first-quarter arrival.
"""

import numpy as np
import ml_dtypes
from contextlib import ExitStack

import concourse.bacc as bacc
import concourse.bass as bass
import concourse.mybir as mybir
import concourse.tile as tile
from concourse.bass import ts, ds
from concourse.bass_utils import run_bass_kernel_spmd
from concourse.masks import make_identity

F32 = mybir.dt.float32
BF16 = mybir.dt.bfloat16
F16 = mybir.dt.float16
FP8 = mybir.dt.float8e4
DR = mybir.MatmulPerfMode.DoubleRow
AF = mybir.ActivationFunctionType
ALU = mybir.AluOpType
AX = mybir.AxisListType

B = 8
NTOK = 4096
C = 512
NH = 8
HD = 64
HID = 2048
NT = 8            # token chunks of 512
TCH = NTOK // NT  # 512 tokens per chunk
P = 128
LN_EPS = 1e-5
SCALE = HD ** -0.5
FC1_FULL8 = True   # fc1 contraction fully fp8-DR (vs half fp8 + half fp16)


def _build(flags):
    """Build the per-core bass program. flags: dict of adaptive bools."""
    nc = bacc.Bacc("TRN2", target_bir_lowering=False, debug=False, num_devices=B)

    x_d = nc.dram_tensor("x", (NTOK, C), F32, kind="ExternalInput").ap()
    # Wk|Wv transposed: [c, 1024] (k columns 0:512 carry the attention scale)
    wkvT_d = nc.dram_tensor("wkvT", (C, 2 * C), F16, kind="ExternalInput").ap()
    # Wq raw (row e, col c'): [512, 512]
    wq_d = nc.dram_tensor("wq", (C, C), F16, kind="ExternalInput").ap()
    projwT_d = nc.dram_tensor("projwT", (C, C), F16, kind="ExternalInput").ap()
    n8 = C if FC1_FULL8 else 2 * P
    fc1w8T_d = nc.dram_tensor("fc1w8T", (n8, HID), FP8, kind="ExternalInput").ap()
    if not FC1_FULL8:
        fc1w16T_d = nc.dram_tensor("fc1w16T", (2 * P, HID), F16,
                                   kind="ExternalInput").ap()
    fc2wT_d = nc.dram_tensor("fc2wT", (HID, C), FP8, kind="ExternalInput").ap()
    fc1b_d = nc.dram_tensor("fc1b", (P, HID // P), F32, kind="ExternalInput").ap()
    fc1s_d = nc.dram_tensor("fc1s", (P, HID // P), F32, kind="ExternalInput").ap()
    fc2b_d = nc.dram_tensor("fc2b", (P, C // P), F32, kind="ExternalInput").ap()
    fc2s_d = nc.dram_tensor("fc2s", (P, C // P), F32, kind="ExternalInput").ap()
    ecaw_d = nc.dram_tensor("ecaw", (1, 3), F32, kind="ExternalInput").ap()
    w2s_d = nc.dram_tensor("w2s", (P, HID // P), FP8, kind="ExternalInput").ap()
    fc2bsn_d = nc.dram_tensor("fc2bsn", (1, 1), F32, kind="ExternalInput").ap()
    if flags["proj_bias"]:
        projb_d = nc.dram_tensor("projb", (1, C), F16, kind="ExternalInput").ap()
    ln_d = {}
    for nm in ("ln1w", "ln1b", "ln2w", "ln2b"):
        if flags[nm]:
            ln_d[nm] = nc.dram_tensor(nm, (C,), F32, kind="ExternalInput").ap()

    out_d = nc.dram_tensor("out", (NTOK, C), F16, kind="ExternalOutput").ap()

    v = nc.vector
    g = nc.gpsimd
    s = nc.scalar
    t = nc.tensor
    sy = nc.sync

    # strided views: token n = 1024*j + 8*p + a  <->  [a][p, j, :]
    x_perm = x_d.rearrange("(j p e) c -> e p j c", e=8, p=P)
    x2_perm = out_d.rearrange("(j p e) c -> e p j c", e=8, p=P)

    with tile.TileContext(nc) as tc, ExitStack() as ctx:
        # ---------------- pools ----------------
        consts = ctx.enter_context(tc.tile_pool(name="consts", bufs=1))
        wpool = ctx.enter_context(tc.tile_pool(name="wpool", bufs=1))
        ctp = ctx.enter_context(tc.tile_pool(name="ctp", bufs=1))
        bnd = ctx.enter_context(tc.tile_pool(name="bnd", bufs=1))
        xin = ctx.enter_context(tc.tile_pool(name="xin", bufs=4))
        curp = ctx.enter_context(tc.tile_pool(name="curp", bufs=5))
        curTp = ctx.enter_context(tc.tile_pool(name="curTp", bufs=2))
        statp = ctx.enter_context(tc.tile_pool(name="statp", bufs=3))
        smp = ctx.enter_context(tc.tile_pool(name="smp", bufs=1))
        x2p = ctx.enter_context(tc.tile_pool(name="x2p", bufs=4))
        h1p = ctx.enter_context(tc.tile_pool(name="h1p", bufs=1))
        yp = ctx.enter_context(tc.tile_pool(name="yp", bufs=1))
        outp = ctx.enter_context(tc.tile_pool(name="outp", bufs=2))

        ps_t = ctx.enter_context(tc.tile_pool(name="ps_t", bufs=2, space="PSUM"))
        ps_mm = ctx.enter_context(tc.tile_pool(name="ps_mm", bufs=5, space="PSUM"))
        ps_sm = ctx.enter_context(tc.tile_pool(name="ps_sm", bufs=1, space="PSUM"))

        # ---------------- phase-1 critical DMAs first ----------------
        # x chunk 0 split into 4 quarters, one per queue, so LN1+COV start on
        # first-quarter arrival; chunk 1 right behind on the same 4 queues.
        qeng = (sy, s, g, sy)
        xbs = {}
        for nt in (0, 1):
            xbs[nt] = xin.tile([P, 4, C], F32, name=f"xb{nt}", tag="xb", bufs=3)
            for q4 in range(4):
                qeng[(q4 + 2 * nt) % 4].dma_start(
                    out=xbs[nt][:, q4, :],
                    in_=x_d[ds(nt * TCH + q4 * P, P), :])
        ident = consts.tile([P, P], F16)
        make_identity(nc, ident)
        ones_colh = consts.tile([P, 1], F16)   # lhsT for partition-sum of y tiles
        v.memset(ones_colh, 1.0)
        ones_row = consts.tile([1, P], F16)    # lhsT for broadcast outer product
        v.memset(ones_row, 1.0)
        # preload the Exp act-table off the critical path (softmax boundary)
        tiny = consts.tile([1, 1], F32)
        v.memset(tiny, 0.0)
        s.activation(out=tiny, in_=tiny, func=AF.Exp)

        ln_bc = {}
        for nm in ln_d:
            bc = wpool.tile([P, C], F32, tag=f"lnbc_{nm}")
            g.dma_start(
                out=bc,
                in_=bass.AP(tensor=ln_d[nm].tensor, offset=ln_d[nm].offset,
                            ap=[[0, P], [1, C]]),
            )
            ln_bc[nm] = bc

        curT_sb = ctp.tile([P, 4, NTOK], F16)
        curT_v = curT_sb.rearrange("p jc (a i) -> p jc a i", a=8)

        def ln_dve(src_tiles, w_bc, b_bc, apply_eng="v"):
            """LayerNorm: DVE stats + rstd; apply on DVE or ScalarE ("s")."""
            n = len(src_tiles)
            mv = statp.tile([P, n, 2], F32, tag="mv", name=f"mv{n}")
            st = statp.tile([P, 6], F32, tag="st6")
            for p in range(n):
                v.bn_stats(out=st, in_=src_tiles[p])
                v.bn_aggr(out=mv[:, p, :], in_=st)
                st = statp.tile([P, 6], F32, tag="st6")
            # rstd = 1/sqrt(var+eps) via DVE reciprocal + 2 Newton steps
            aN = statp.tile([P, n], F32, tag="veps", name=f"veps{n}")
            v.tensor_scalar_add(out=aN, in0=mv[:, :, 1], scalar1=LN_EPS)
            rstd = statp.tile([P, n], F32, tag="rstd", name=f"rstd{n}")
            v.reciprocal(out=rstd, in_=aN)
            tN = statp.tile([P, n], F32, tag="tN", name=f"tN{n}")
            uN = statp.tile([P, n], F32, tag="uN", name=f"uN{n}")
            for _ in range(2):
                v.tensor_mul(out=tN, in0=rstd, in1=rstd)
                v.tensor_mul(out=tN, in0=tN, in1=aN)
                v.tensor_scalar(out=uN, in0=tN, scalar1=-0.5, scalar2=1.5,
                                op0=ALU.mult, op1=ALU.add)
                v.tensor_mul(out=rstd, in0=rstd, in1=uN)
            nmr = None
            if apply_eng == "s":
                # scalar-engine apply: out = Identity(x*rstd + (-mu*rstd))
                nmr = statp.tile([P, n], F32, tag="nmr", name=f"nmr{n}")
                v.tensor_mul(out=nmr, in0=mv[:, :, 0], in1=rstd)
                v.tensor_scalar_mul(out=nmr, in0=nmr, scalar1=-1.0)
            curs = []
            for p in range(n):
                if w_bc is None and b_bc is None:
                    cur = curp.tile([P, TCH], F16, tag="cur", bufs=12)
                    if apply_eng == "s":
                        s.activation(out=cur, in_=src_tiles[p], func=AF.Identity,
                                     bias=nmr[:, p:p + 1], scale=rstd[:, p:p + 1])
                    else:
                        v.tensor_scalar(out=cur, in0=src_tiles[p],
                                        scalar1=mv[:, p, 0:1],
                                        scalar2=rstd[:, p:p + 1],
                                        op0=ALU.subtract, op1=ALU.mult)
                else:
                    tmp = curp.tile([P, TCH], F32, tag="curf")
                    v.tensor_scalar(out=tmp, in0=src_tiles[p],
                                    scalar1=mv[:, p, 0:1], scalar2=rstd[:, p:p + 1],
                                    op0=ALU.subtract, op1=ALU.mult)
                    cur = curp.tile([P, TCH], F16, tag="cur", bufs=12)
                    if w_bc is not None and b_bc is not None:
                        v.tensor_mul(out=tmp, in0=tmp, in1=w_bc)
                        v.tensor_add(out=cur, in0=tmp, in1=b_bc)
                    elif w_bc is not None:
                        v.tensor_mul(out=cur, in0=tmp, in1=w_bc)
                    else:
                        v.tensor_add(out=cur, in0=tmp, in1=b_bc)
                curs.append(cur)
            return curs

        # ================= PHASE 1: LN1 + COV + curT (sw-pipelined) ======
        cov_ps = [ps_mm.tile([P, TCH], F32, tag="mm", name=f"cov{ci}")
                  for ci in range(4)]

        # COV is symmetric: accumulate only upper-triangle blocks
        # (bank ci covers columns [128*ci, 512)); lower blocks are
        # reconstructed by transposition at the boundary.
        def cov_mms(curs, nt):
            if nt == NT - 1:
                # last chunk ci-major so bank ci completes early for the copy
                for ci in range(4):
                    for p in range(4):
                        t.matmul(cov_ps[ci][:, ds(P * ci, C - P * ci)],
                                 lhsT=curs[p][:, ts(ci, P)],
                                 rhs=curs[p][:, ds(P * ci, C - P * ci)],
                                 start=False, stop=(p == 3),
                                 skip_group_check=True)
            else:
                for p in range(len(curs)):
                    for ci in range(4):
                        t.matmul(cov_ps[ci][:, ds(P * ci, C - P * ci)],
                                 lhsT=curs[p][:, ts(ci, P)],
                                 rhs=curs[p][:, ds(P * ci, C - P * ci)],
                                 start=False, stop=False, skip_group_check=True)

        def cov_mms_q(cur, first):
            for ci in range(4):
                t.matmul(cov_ps[ci][:, ds(P * ci, C - P * ci)],
                         lhsT=cur[:, ts(ci, P)],
                         rhs=cur[:, ds(P * ci, C - P * ci)],
                         start=first, stop=False, skip_group_check=True)

        def transp_scatter(nt, curs):
            """Transpose 4 cur tiles, scatter into permuted curT (fp16)."""
            for cj in range(4):
                pst = ps_t.tile([P, TCH], F32, tag="pst")
                for p in range(4):
                    t.matmul(pst[:, ts(p, P)], lhsT=curs[p][:, ts(cj, P)],
                             rhs=ident, start=True, stop=True)
                # permuted write: curT[:, cj, a*512 + 64*nt + i] = cur[8i+a]
                dst = curT_v[:, cj, :, ds(64 * nt, 64)]
                src = pst.rearrange("p (i a) -> p a i", a=8)
                if cj == 1:
                    v.tensor_copy(out=dst, in_=src)
                else:
                    s.copy(out=dst, in_=src)

        for nt in range(NT):
            xb = xbs.pop(nt)
            xts = [xb[:, q, :] for q in range(4)]
            if nt == 0:
                curs = []
                for q in range(4):
                    cq = ln_dve([xts[q]], ln_bc.get("ln1w"), ln_bc.get("ln1b"),
                                apply_eng="s")
                    cov_mms_q(cq[0], first=(q == 0))
                    curs.append(cq[0])
            else:
                curs = ln_dve(xts, ln_bc.get("ln1w"), ln_bc.get("ln1b"),
                              apply_eng="s")
                cov_mms(curs, nt)
            if nt == NT - 1:
                curs_last = curs  # transposed after the boundary A MMs
            else:
                transp_scatter(nt, curs)
            # prefetch AFTER this chunk's work so the scheduler orders this
            # chunk's LN ahead of the next chunks' DMA-dependent stats
            pres = (2, 3, 4) if nt == 0 else ((nt + 4,) if nt + 4 < NT else ())
            for pre in pres:
                xbs[pre] = xin.tile([P, 4, C], F32, name=f"xb{nt}_{pre}",
                                    tag="xb", bufs=3)
                sy.dma_start(
                    out=xbs[pre],
                    in_=x_d[ds(pre * TCH, TCH), :].rearrange(
                        "(q p) c -> p q c", p=P))
            if nt == 1:
                # prefetch all weights (g/v/s queues; sy keeps feeding x)
                wkv_sb = wpool.tile([P, 4, 2 * C], F16)
                for cj in range(4):
                    (g if cj % 2 else s).dma_start(out=wkv_sb[:, cj, :],
                                                   in_=wkvT_d[ts(cj, P), :])
                wq_sb = wpool.tile([P, 4, C], F16)
                g.dma_start(out=wq_sb,
                            in_=wq_d[:, :].rearrange("(q p) c -> p q c", p=P))
                projw_sb = wpool.tile([P, 4, C], F16)
                s.dma_start(out=projw_sb,
                            in_=projwT_d[:, :].rearrange("(q p) c -> p q c", p=P))
                fc1w8_sb = wpool.tile([P, n8 // P, HID], FP8)
                g.dma_start(out=fc1w8_sb,
                            in_=fc1w8T_d[:, :].rearrange("(q p) c -> p q c", p=P))
                if not FC1_FULL8:
                    fc1w16_sb = wpool.tile([P, 2, HID], F16)
                    for q2 in range(2):
                        (g if q2 else s).dma_start(
                            out=fc1w16_sb[:, q2, :], in_=fc1w16T_d[ts(q2, P), :])
                fc2w_sb = wpool.tile([P, 16, C], FP8)
                for jc in range(4):
                    (s if jc % 2 else g).dma_start(
                        out=fc2w_sb[:, ts(jc, 4), :],
                        in_=fc2wT_d[ds(jc * 4 * P, 4 * P), :].rearrange(
                            "(q p) c -> p q c", p=P))
                fc1b_sb = wpool.tile([P, HID // P], F32)
                s.dma_start(out=fc1b_sb, in_=fc1b_d[:, :])
                fc1s_sb = wpool.tile([P, HID // P], F32)
                s.dma_start(out=fc1s_sb, in_=fc1s_d[:, :])
                fc2b_sb = wpool.tile([P, C // P], F32)
                s.dma_start(out=fc2b_sb, in_=fc2b_d[:, :])
                fc2s_sb = wpool.tile([P, C // P], F32)
                s.dma_start(out=fc2s_sb, in_=fc2s_d[:, :])
                eca_sb = wpool.tile([1, 3], F32)
                s.dma_start(out=eca_sb, in_=ecaw_d[:, :])
                w2s_sb = wpool.tile([P, HID // P], FP8)
                s.dma_start(out=w2s_sb, in_=w2s_d[:, :])
                fc2bsn_sb = wpool.tile([1, 1], F32)
                s.dma_start(out=fc2bsn_sb, in_=fc2bsn_d[:, :])
                if flags["proj_bias"]:
                    projb_sb = wpool.tile([1, C], F16)
                    s.dma_start(out=projb_sb, in_=projb_d[:, :])

        # ================= BOUNDARY: COV -> A -> logits -> softmax -> G -> WqG
        def cpy(eng, out, in_):
            if eng is s:
                s.copy(out=out, in_=in_)
            else:
                eng.tensor_copy(out=out, in_=in_)

        cov_sb = bnd.tile([P, 4, C], F16)
        for ci in range(4):
            cpy(v if ci % 2 else s,
                cov_sb[:, ci, ds(P * ci, C - P * ci)],
                cov_ps[ci][:, ds(P * ci, C - P * ci)])
        # lower-triangle blocks: COV[cj-rows, ci-cols] = COV[ci-rows, cj-cols]^T
        for ci in range(4):
            for cj in range(ci + 1, 4):
                pstx = ps_t.tile([P, TCH], F32, tag="pst")
                t.matmul(pstx[:, 0:P], lhsT=cov_sb[:, ci, ds(P * cj, P)],
                         rhs=ident, start=True, stop=True)
                cpy(v if (ci + cj) % 2 else s,
                    cov_sb[:, cj, ds(P * ci, P)], pstx[:, 0:P])
        a_ps = [ps_mm.tile([P, C], F32, tag="mm", name=f"aps{ci}")
                for ci in range(4)]
        for ci in range(4):
            for cj in range(4):
                t.matmul(a_ps[ci], lhsT=cov_sb[:, cj, ds(ci * P, P)],
                         rhs=wkv_sb[:, cj, ds(C, C)],
                         start=(cj == 0), stop=(cj == 3))
        a_sb = bnd.tile([P, 4, C], F16)
        for ci in range(4):
            cpy(v if ci % 2 else s, a_sb[:, ci, :], a_ps[ci])
        # chunk 7's transposes deferred here: they overlap the softmax/G/WqG
        # DVE+ACT chain instead of delaying the A matmuls
        transp_scatter(NT - 1, curs_last)

        logits_ps = ps_sm.tile([P, 4, HD], F32, tag="psm")
        for h in range(NH):
            hp, half = h // 2, h % 2
            rows = slice(64 * half, 64 * half + 64)
            for cj in range(4):
                t.matmul(logits_ps[rows, hp, :],
                         lhsT=wkv_sb[:, cj, ds(64 * h, 64)],
                         rhs=a_sb[:, cj, ds(64 * h, 64)],
                         start=(cj == 0), stop=(cj == 3))

        # softmax + G + WqG accumulation, pipelined per head-pair
        wqg_ps = [ps_mm.tile([P, C], F32, tag="mm", name=f"wqg{ci}")
                  for ci in range(4)]
        G_sb = bnd.tile([P, 4, C], F16)
        for hp in range(4):
            a128 = smp.tile([P, P], F16, tag="a128", bufs=2)
            for half in range(2):
                rows = slice(64 * half, 64 * half + 64)
                nm = smp.tile([P, 1], F32, tag="nm", bufs=2)
                v.tensor_reduce(out=nm[rows, :], in_=logits_ps[rows, hp, :],
                                axis=AX.X, op=ALU.max, negate=True)
                esb = smp.tile([P, 64], F32, tag="esb", bufs=2)
                ssum = smp.tile([P, 1], F32, tag="ssum", bufs=2)
                s.activation(out=esb[rows, :], in_=logits_ps[rows, hp, :],
                             func=AF.Exp, bias=nm[rows, :], scale=1.0,
                             accum_out=ssum[rows, :])
                v.reciprocal(out=ssum[rows, :], in_=ssum[rows, :])
                v.tensor_scalar_mul(out=a128[rows, ds(64 * half, 64)],
                                    in0=esb[rows, :], scalar1=ssum[rows, :])
            # G_h[e, c] = sum_d attn_h[d, e] * projwT[64h+d, c]
            gps = ps_t.tile([P, TCH], F32, tag="pst")
            for half in range(2):
                rows = slice(64 * half, 64 * half + 64)
                t.matmul(gps[rows, 0:C], lhsT=a128[rows, rows],
                         rhs=projw_sb[rows, hp, :], start=True, stop=True)
            s.copy(out=G_sb[:, hp, :], in_=gps[:, 0:C])
            # WqG[c', c] += Wq[e-block hp, c']^T @ G[e-block hp, c]
            for ci in range(4):
                t.matmul(wqg_ps[ci], lhsT=wq_sb[:, hp, ds(ci * P, P)],
                         rhs=G_sb[:, hp, :],
                         start=(hp == 0), stop=(hp == 3), skip_group_check=True)
        wqg_sb = bnd.tile([P, 4, C], F16)
        for ci in range(4):
            cpy(v if ci % 2 else s, wqg_sb[:, ci, :], wqg_ps[ci])

        # ================= PHASE 2+3: x2 = cur@WqG (+x), LN2, MLP (permuted) ==
        pool_ps = ps_sm.tile([1, C], F32, tag="psm")
        yT_sb = yp.tile([P, 4, NTOK], F16)

        def fc1_jc(jc, cur2T8, cur2T16, h1T):
            ps = ps_mm.tile([P, TCH], F32, tag="mm")
            if FC1_FULL8:
                for k in range(2):
                    t.matmul(ps, lhsT=fc1w8_sb[:, 2 * k:2 * k + 2, ts(jc, P)],
                             rhs=cur2T8[:, 2 * k:2 * k + 2, :],
                             start=(k == 0), stop=(k == 1), perf_mode=DR)
            else:
                # half the contraction (channels 0-255) in fp8 DoubleRow
                t.matmul(ps, lhsT=fc1w8_sb[:, 0:2, ts(jc, P)],
                         rhs=cur2T8[:, 0:2, :],
                         start=True, stop=False, perf_mode=DR)
                for k in range(2):
                    t.matmul(ps, lhsT=fc1w16_sb[:, k, ts(jc, P)],
                             rhs=cur2T16[:, k, :],
                             start=False, stop=(k == 1))
            s.activation(out=h1T[:, jc, :], in_=ps, func=AF.Gelu,
                         bias=fc1b_sb[:, jc:jc + 1],
                         scale=fc1s_sb[:, jc:jc + 1])

        def fc2_y(a, cc, ps):
            yslc = yT_sb[:, cc, ds(a * TCH, TCH)]
            v.tensor_scalar(out=yslc, in0=ps,
                            scalar1=fc2s_sb[:, cc:cc + 1],
                            scalar2=fc2b_sb[:, cc:cc + 1],
                            op0=ALU.mult, op1=ALU.add)
            return yslc

        def fc_block(a, cur2T8, cur2T16):
            """fc1 + fc2 with fc2's cc 0-1 interleaved k-major into fc1's
            gelu-paced stretch, so the PE isn't starved while ScalarE drains
            GELUs.  Last chunk interleaves the h1 pool matmuls instead (gate
            chain latency beats fc2 there)."""
            h1T = h1p.tile([P, 16, TCH], FP8, tag="h1T")
            last = a == NT - 1
            fc2ps = [ps_t.tile([P, TCH], F32, tag="pst", name=f"fc2ps{cc}")
                     for cc in range(2)]

            def fc2_pair(k, stop):
                for cc in range(2):
                    t.matmul(fc2ps[cc],
                             lhsT=fc2w_sb[:, 2 * k:2 * k + 2, ts(cc, P)],
                             rhs=h1T[:, 2 * k:2 * k + 2, :],
                             start=(k == 0), stop=stop, perf_mode=DR,
                             skip_group_check=True)

            def pool_h1(jc, stop):
                t.matmul(pool_ps[0:1, :], lhsT=w2s_sb[:, jc:jc + 1],
                         rhs=h1T[:, jc, :],
                         start=False, stop=stop, skip_group_check=True)

            for jc in range(4):
                fc1_jc(jc, cur2T8, cur2T16, h1T)
            for k in range(6):
                if last:
                    pool_h1(2 * k, False)
                    pool_h1(2 * k + 1, False)
                else:
                    fc2_pair(k, stop=False)
                fc1_jc(4 + 2 * k, cur2T8, cur2T16, h1T)
                fc1_jc(5 + 2 * k, cur2T8, cur2T16, h1T)
            if last:
                for jc in range(12, 16):
                    pool_h1(jc, stop=(jc == 15))
                for k in range(8):
                    fc2_pair(k, stop=(k == 7))
            else:
                fc2_pair(6, stop=False)
                fc2_pair(7, stop=True)
            ys = [fc2_y(a, cc, fc2ps[cc]) for cc in range(2)]
            for cc in range(2, 4):
                ps = ps_t.tile([P, TCH], F32, tag="pst", name=f"fc2ps{cc}")
                for k in range(8):
                    t.matmul(ps, lhsT=fc2w_sb[:, 2 * k:2 * k + 2, ts(cc, P)],
                             rhs=h1T[:, 2 * k:2 * k + 2, :],
                             start=(k == 0), stop=(k == 7), perf_mode=DR)
                ys.append(fc2_y(a, cc, ps))
            if not last:
                # pooled[i] += sum_ch y[8i+a, ch]: pre-sum the 4 cc tiles on
                # DVE, then ONE partition-reduction matmul
                ysum = x2p.tile([P, TCH], F16, tag="ysum", bufs=2)
                v.tensor_add(out=ysum, in0=ys[0], in1=ys[1])
                v.tensor_add(out=ysum, in0=ysum, in1=ys[2])
                v.tensor_add(out=ysum, in0=ysum, in1=ys[3])
                t.matmul(pool_ps[0:1, :], lhsT=ones_colh, rhs=ysum,
                         start=(a == 0), stop=False, skip_group_check=True)

        def ln_pe2(curs):
            """Transpose LN2 tiles -> fp8 blocks for DR fc1 (+fp16 if half)."""
            cur2T8 = curTp.tile([P, 4 if FC1_FULL8 else 2, TCH], FP8, tag="c2T8")
            cur2T16 = None
            if not FC1_FULL8:
                cur2T16 = curTp.tile([P, 2, TCH], F16, tag="c2T16")
            for cj in range(4):
                pst = ps_t.tile([P, TCH], F32, tag="pst")
                for p in range(4):
                    t.matmul(pst[:, ts(p, P)], lhsT=curs[p][:, ts(cj, P)],
                             rhs=ident, start=True, stop=True)
                if FC1_FULL8:
                    if cj % 2:
                        v.tensor_copy(out=cur2T8[:, cj, :], in_=pst)
                    else:
                        s.copy(out=cur2T8[:, cj, :], in_=pst)
                elif cj < 2:
                    v.tensor_copy(out=cur2T8[:, cj, :], in_=pst)
                else:
                    s.copy(out=cur2T16[:, cj - 2, :], in_=pst)
            return cur2T8, cur2T16

        pend2 = None
        # tail readback tiles; blocks 4-7 reuse the (dead) boundary tiles'
        # SBUF slots -- same [P, 4, C] f16 shape, zero extra space
        xzs = {}
        bnd_tags = {4: "cov_sb", 5: "a_sb", 6: "G_sb", 7: "wqg_sb"}
        for a in range(NT):
            x2ts = []
            for j in range(4):
                ps = ps_mm.tile([P, TCH], F32, tag="mm")
                for cj in range(4):
                    t.matmul(ps, lhsT=curT_sb[:, cj, ds(a * TCH + j * P, P)],
                             rhs=wqg_sb[:, cj, :],
                             start=(cj == 0),
                             stop=(cj == 3 and not flags["proj_bias"]))
                if flags["proj_bias"]:
                    t.matmul(ps, lhsT=ones_row, rhs=projb_sb,
                             start=False, stop=True)
                xt = xin.tile([P, C], F32, tag="xt", bufs=3)
                sy.dma_start(out=xt, in_=x_perm[a][:, j, :])
                x2t = x2p.tile([P, C], F16, tag="x2t")
                v.tensor_add(out=x2t, in0=ps, in1=xt)
                sy.dma_start(out=x2_perm[a][:, j, :], in_=x2t)
                x2ts.append(x2t)
            if a == NT - 1:
                # x2 fully written once these 4 land: issue all tail readbacks
                # now so they drain during the last two fc_blocks
                for ra in range(NT):
                    if ra < 4:
                        xz = outp.tile([P, 4, C], F16, name=f"xz{ra}",
                                       tag="xz", bufs=4)
                    else:
                        xz = bnd.tile([P, 4, C], F16, name=f"xz{ra}",
                                      tag=bnd_tags[ra])
                    (sy, s, g)[ra % 3].dma_start(
                        out=xz,
                        in_=out_d[ds(ra * TCH, TCH), :].rearrange(
                            "(cc p) c -> p cc c", p=P))
                    xzs[ra] = xz
            curs = ln_dve(x2ts, ln_bc.get("ln2w"), ln_bc.get("ln2b"))
            if pend2 is not None:
                fc_block(*pend2)
            cur2T8, cur2T16 = ln_pe2(curs)
            pend2 = (a, cur2T8, cur2T16)
        # x2 fully written: issue readbacks for the DVE-add tail blocks now so
        # they land during the final fc_block while DMA is otherwise idle
        fc_block(*pend2)

        # ================= TAIL =================
        # ----- ECA gate -----
        ppad = smp.tile([1, C + 2], F32, tag="ppad")
        v.memset(ppad, 0.0)
        s.activation(out=ppad[:, 1:C + 1], in_=pool_ps, func=AF.Identity,
                     bias=fc2bsn_sb[0:1, 0:1], scale=1.0 / NTOK)
        cv = smp.tile([1, C], F32, tag="cv")
        v.tensor_scalar_mul(out=cv, in0=ppad[0:1, 0:C], scalar1=eca_sb[0:1, 0:1])
        v.scalar_tensor_tensor(out=cv, in0=ppad[0:1, 1:C + 1], scalar=eca_sb[0:1, 1:2],
                               in1=cv, op0=ALU.mult, op1=ALU.add)
        v.scalar_tensor_tensor(out=cv, in0=ppad[0:1, 2:C + 2], scalar=eca_sb[0:1, 2:3],
                               in1=cv, op0=ALU.mult, op1=ALU.add)
        cvb = smp.tile([1, C], F16, tag="cvb")
        s.activation(out=cvb, in_=cv, func=AF.Sigmoid)
        psb = ps_t.tile([P, TCH], F32, tag="pst")
        t.matmul(psb[:, 0:C], lhsT=ones_row, rhs=cvb, start=True, stop=True)
        sB = consts.tile([P, C], F16)
        # +1 (the "y4*gate + y4" residual) folded into the broadcast evac
        s.activation(out=sB, in_=psb[:, 0:C], func=AF.Identity, bias=1.0)

        # ----- out[512a+.] = x2 + sB * yT_a -----
        # gate-muls on DVE for all blocks; the +x2 adds split between the
        # (otherwise idle) PE -- identity-matmul accumulate, ScalarE evac back
        # into the readback tile -- and DVE block-wide adds.
        for a in range(NT):
            wt = outp.tile([P, 4, C], F16, tag="wt", bufs=2)
            for cc in range(4):
                v.tensor_mul(out=wt[:, cc, :],
                             in0=yT_sb[:, cc, ds(a * TCH, TCH)], in1=sB)
            if a < 4:
                for cc in range(4):
                    zps = ps_mm.tile([P, TCH], F32, tag="mm")
                    t.matmul(zps, lhsT=ident, rhs=wt[:, cc, :],
                             start=True, stop=False)
                    t.matmul(zps, lhsT=ident, rhs=xzs[a][:, cc, :],
                             start=False, stop=True)
                    s.copy(out=xzs[a][:, cc, :], in_=zps)
            else:
                v.tensor_add(out=xzs[a], in0=xzs[a], in1=wt)
            (sy, s, g)[a % 3].dma_start(
                out=out_d[ds(a * TCH, TCH), :].rearrange(
                    "(cc p) c -> p cc c", p=P),
                in_=xzs[a])

    nc.compile()
    return nc


_CACHE = {}


def _get_program(flags):
    key = tuple(sorted(flags.items()))
    if key not in _CACHE:
        _CACHE[key] = _build(flags)
    return _CACHE[key]


def _host_prep(inputs):
    f8 = ml_dtypes.float8_e4m3
    qkv_w = np.asarray(inputs["qkv_w"], np.float32).copy()
    qkv_w[C:2 * C, :] *= SCALE  # fold attention scale into k weights
    flags = {
        "ln1w": not np.all(inputs["ln1_w"] == 1.0),
        "ln1b": bool(np.any(inputs["ln1_b"] != 0.0)),
        "ln2w": not np.all(inputs["ln2_w"] == 1.0),
        "ln2b": bool(np.any(inputs["ln2_b"] != 0.0)),
        "proj_bias": bool(np.any(inputs["proj_b"] != 0.0)),
    }
    fc1w = np.asarray(inputs["fc1_w"], np.float32)          # (HID, C)
    fc2w = np.asarray(inputs["fc2_w"], np.float32)          # (C, HID)
    s1 = 224.0 / np.maximum(np.abs(fc1w).max(axis=1), 1e-6)  # per hidden j
    s2 = 224.0 / np.maximum(np.abs(fc2w).max(axis=1), 1e-6)  # per out-ch c
    fc1ws = fc1w * s1[:, None]
    fc2ws = fc2w * s2[:, None]
    common = {
        "wkvT": np.ascontiguousarray(qkv_w[C:3 * C].T).astype(np.float16),
        "wq": np.ascontiguousarray(qkv_w[0:C]).astype(np.float16),
        "projwT": np.ascontiguousarray(
            np.asarray(inputs["proj_w"], np.float32).T).astype(np.float16),
        "fc1w8T": np.ascontiguousarray(
            fc1ws.T if FC1_FULL8 else fc1ws.T[:2 * P, :]).astype(f8),
        "fc2wT": np.ascontiguousarray(fc2ws.T).astype(f8),
        **({} if FC1_FULL8 else {
            "fc1w16T": np.ascontiguousarray(fc1ws.T[2 * P:, :])
            .astype(np.float16)}),
        "fc1b": np.ascontiguousarray(
            np.asarray(inputs["fc1_b"], np.float32).reshape(HID // P, P).T),
        "fc1s": np.ascontiguousarray((1.0 / s1).reshape(HID // P, P).T),
        "fc2b": np.ascontiguousarray(
            np.asarray(inputs["fc2_b"], np.float32).reshape(C // P, P).T),
        "fc2s": np.ascontiguousarray((1.0 / s2).reshape(C // P, P).T),
        "ecaw": np.asarray(inputs["eca_w"], np.float32).reshape(1, 3),
        "w2s": np.ascontiguousarray(
            fc2w.sum(axis=0).reshape(HID // P, P).T).astype(f8),
        "fc2bsn": np.asarray(inputs["fc2_b"], np.float32).sum()
            .reshape(1, 1) / NTOK,
    }
    if flags["proj_bias"]:
        common["projb"] = np.asarray(inputs["proj_b"], np.float32) \
            .reshape(1, C).astype(np.float16)
    for nm, key in (("ln1w", "ln1_w"), ("ln1b", "ln1_b"),
                    ("ln2w", "ln2_w"), ("ln2b", "ln2_b")):
        if flags[nm]:
            common[nm] = np.asarray(inputs[key], np.float32)
    return flags, common


def kernel(**inputs):
    flags, common = _host_prep(inputs)
    nc = _get_program(flags)
    x = np.asarray(inputs["x"], np.float32)
    in_maps = [dict(common, x=np.ascontiguousarray(x[i])) for i in range(B)]
    res = run_bass_kernel_spmd(nc, in_maps, list(range(B)))
    return np.stack([np.asarray(r["out"], np.float32) for r in res.results], axis=0)
